# revision 14
# baseline (speedup 1.0000x reference)
"""Compact Bilinear Pooling (count-sketch + FFT circular correlation) as a
Trainium2 Bass kernel, data-parallel over batch across 8 NeuronCores.

Math: FFT(count_sketch(x; s, h))[k] = sum_c x[c] * s[c] * exp(-2pi i h[c] k / D)
    = x @ A, a dense complex matrix built on the host from (s, h). So the whole
layer is: Y1 = X1 @ A1, Y2 = X2 @ A2 (per-row half spectra), elementwise
complex product fused with the 14x14 sum-pool (DVE tensor_tensor_reduce),
then a real inverse FFT of the pooled [4, D] spectrum per core, done as a
two-stage Cooley-Tukey factorization (D = 64*128) of small matmuls.

The big spectrum matmuls run in bf16 (fast weight load + half the HBM
traffic); accumulation stays fp32 in PSUM. The pooled spectrum is reassembled
for the IFFT with PE transposes (no DRAM bounce).
"""
import numpy as np
import ml_dtypes

import concourse.bass as bass
import concourse.tile as tile
from concourse import bacc, mybir
from concourse.bass_utils import run_bass_kernel_spmd

B, Hh, Ww, C, D = 32, 14, 14, 512, 8192
NCORES = 8
BPC = B // NCORES        # 4 batches per core
HW = Hh * Ww             # 196
ROWS = BPC * HW          # 784 rows per core
KT = 33                  # frequency tiles of 128
KP = KT * 128            # 4224 >= D/2 + 1
CCN = 4                  # contraction chunks (C = 4*128)

F32 = mybir.dt.float32
BF16 = mybir.dt.bfloat16


def _build_nc():
    nc = bacc.Bacc("TRN2", target_bir_lowering=False)

    xt_d = nc.dram_tensor("xt", [128, 2, CCN, ROWS], BF16, kind="ExternalInput")
    amat_d = nc.dram_tensor("amat", [128, 4, CCN, KT, 128], BF16, kind="ExternalInput")
    w1_d = nc.dram_tensor("w1", [128, 3, 128], F32, kind="ExternalInput")
    w2_d = nc.dram_tensor("w2", [64, 2, 64], F32, kind="ExternalInput")
    tw_d = nc.dram_tensor("tw", [64, 2, 128], F32, kind="ExternalInput")
    id_d = nc.dram_tensor("ident", [128, 128], F32, kind="ExternalInput")
    out_d = nc.dram_tensor("out", [BPC, D], F32, kind="ExternalOutput")

    with tile.TileContext(nc) as tc:
        with tc.tile_pool(name="const", bufs=1) as pc, \
             tc.tile_pool(name="astream", bufs=3) as pa, \
             tc.tile_pool(name="ysbp", bufs=2) as pysb, \
             tc.tile_pool(name="scr", bufs=2) as pscr, \
             tc.tile_pool(name="qstage", bufs=1) as pqs:

            xt = pc.tile([128, 2, CCN, ROWS], BF16)
            for cc in range(CCN):
                for inp in range(2):
                    nc.sync.dma_start(xt[:, inp, cc], xt_d[:, inp, cc])
            qsb = pc.tile([128, KT, 2, 4, 2], F32)  # (kt, rc, term, seg)
            qsb_f = qsb  # noqa
            w1 = pc.tile([128, 3, 128], F32)
            nc.sync.dma_start(w1, w1_d[:, :, :])
            w2 = pc.tile([64, 2, 64], F32)
            nc.sync.dma_start(w2, w2_d[:, :, :])
            tw = pc.tile([64, 2, 128], F32)
            nc.sync.dma_start(tw, tw_d[:, :, :])
            ident = pc.tile([128, 128], F32)
            nc.sync.dma_start(ident, id_d[:, :])

            # ---------------- main loop ----------------
            with tc.tile_pool(name="py", bufs=2, space="PSUM") as py:
                for kt in range(KT):
                    at = pa.tile([128, 4, CCN, 128], BF16, tag="amat")
                    for t_ in range(4):
                        nc.sync.dma_start(at[:, t_], amat_d[:, t_, :, kt, :])
                    for rc in range(2):
                        # two 2-bank PSUM tiles: [re at 0:392 | im at 512:904]
                        yps = {}
                        for inp in range(2):
                            yps[inp] = py.tile([128, 1024], F32, tag=f"y{inp}",
                                               name=f"y{inp}")
                        for cc in range(CCN):
                            for inp in range(2):
                                for ri in range(2):
                                    nc.tensor.matmul(
                                        yps[inp][:, ri * 512:ri * 512 + 392],
                                        at[:, inp * 2 + ri, cc, :],
                                        xt[:, inp, cc, rc * 392:(rc + 1) * 392],
                                        start=(cc == 0),
                                        stop=(cc == CCN - 1),
                                    )
                        # one strided ACT copy per input: PSUM fp32 -> SBUF bf16
                        ysb = {}
                        for inp in range(2):
                            ysb[inp] = pysb.tile([128, 2, 392], BF16,
                                                 tag=f"ysb{inp}", name=f"ysb{inp}")
                            nc.scalar.copy(
                                ysb[inp],
                                yps[inp].rearrange("p (s x) -> p s x", s=2)[:, :, :392])
                        # products on DVE at 2x (bf16 SBUF x bf16 SBUF)
                        # (0: U=y1r*y2r, 1: V=y1i*y2i, 2: T1=y1r*y2i, 3: T2=y1i*y2r)
                        prods = pscr.tile([128, 4, 392], BF16, tag="prods",
                                          name="prods")
                        for term, (i0, i1) in enumerate([(0, 0), (1, 1), (0, 1), (1, 0)]):
                            nc.vector.tensor_mul(prods[:, term],
                                                 ysb[0][:, i0], ysb[1][:, i1])
                        # pooling: GPSIMD pre-folds 196 -> 98, then one
                        # multi-output free-axis reduce on DVE
                        pv = prods.rearrange("p t (s x) -> p t s x", s=2)
                        pref = pscr.tile([128, 4, 2, 98], F32, tag="pref",
                                         name="pref")
                        nc.gpsimd.tensor_tensor(
                            pref, pv[:, :, :, 0:98], pv[:, :, :, 98:196],
                            mybir.AluOpType.add)
                        nc.vector.tensor_reduce(
                            qsb[:, kt, rc].rearrange("p t s -> p (t s)"),
                            pref.rearrange("p t s x -> p (t s) x"),
                            axis=mybir.AxisListType.X, op=mybir.AluOpType.add)

            # combine terms: Qr = U - V, Qi = T1 + T2
            qstf = pqs.tile([128, KT, 2, 4], F32, tag="qstf")
            qstf_v = qstf.rearrange("p kt t (rc s) -> p kt t rc s", rc=2)
            nc.vector.tensor_sub(qstf_v[:, :, 0], qsb[:, :, :, 0, :], qsb[:, :, :, 1, :])
            nc.vector.tensor_add(qstf_v[:, :, 1], qsb[:, :, :, 2, :], qsb[:, :, :, 3, :])

            # ---------------- inverse FFT tail ----------------
            with tc.tile_pool(name="pif", bufs=1, space="PSUM") as pif, \
                 tc.tile_pool(name="ptr", bufs=2, space="PSUM") as ptr, \
                 tc.tile_pool(name="ptw", bufs=1, space="PSUM") as ptw, \
                 tc.tile_pool(name="pifs", bufs=1) as pifs, \
                 tc.tile_pool(name="ptmp", bufs=2) as ptmp:
                # reassemble Q for stage 1 with PE transposes (no DRAM bounce):
                # qar[a', b*64 + r] = Q[k] with k = kt*128 + p, p = p_hi*64 + r,
                # a' = p_hi*64 + kt (w1 rows are host-permuted to match; rows
                # with kt >= 33 are zero so garbage SBUF there is harmless).
                qar = pifs.tile([128, BPC * 64], F32, tag="qar")
                qai = pifs.tile([128, BPC * 64], F32, tag="qai")
                nc.vector.memset(qar, 0.0)
                nc.vector.memset(qai, 0.0)
                for t, dst in ((0, qar), (1, qai)):
                    for b in range(BPC):
                        tp = ptr.tile([33, 128], F32, tag="tp")
                        nc.tensor.transpose(tp, qstf[:, :, t, b], ident)
                        for ph in range(2):
                            nc.scalar.copy(
                                dst[ph * 64:ph * 64 + 33, b * 64:(b + 1) * 64],
                                tp[:, ph * 64:(ph + 1) * 64])

                # stage 1: V[q, (b,r)] ; contraction over a'
                vr_ps = pif.tile([128, BPC * 64], F32, tag="vr")
                vi_ps = pif.tile([128, BPC * 64], F32, tag="vi")
                nc.tensor.matmul(vr_ps, w1[:, 0, :], qar, start=True, stop=False)
                nc.tensor.matmul(vr_ps, w1[:, 2, :], qai, start=False, stop=True)
                nc.tensor.matmul(vi_ps, w1[:, 1, :], qar, start=True, stop=False)
                nc.tensor.matmul(vi_ps, w1[:, 0, :], qai, start=False, stop=True)
                vr_sb = pifs.tile([128, BPC * 64], F32, tag="vr_sb")
                vi_sb = pifs.tile([128, BPC * 64], F32, tag="vi_sb")
                nc.scalar.copy(vr_sb, vr_ps)
                nc.scalar.copy(vi_sb, vi_ps)

                # per-b transpose [128 q, 64 r] -> [64 r, 128 q], then twiddle
                tr_sb = pifs.tile([64, BPC * 128], F32, tag="tr_sb")
                ti_sb = pifs.tile([64, BPC * 128], F32, tag="ti_sb")
                for b in range(BPC):
                    trp = ptw.tile([64, 128], F32, tag="trp")
                    tip = ptw.tile([64, 128], F32, tag="tip")
                    nc.tensor.transpose(trp, vr_sb[:, b * 64:(b + 1) * 64], ident)
                    nc.tensor.transpose(tip, vi_sb[:, b * 64:(b + 1) * 64], ident)
                    m1 = ptmp.tile([64, 128], F32, tag="m1")
                    m2 = ptmp.tile([64, 128], F32, tag="m2")
                    m3 = ptmp.tile([64, 128], F32, tag="m3")
                    m4 = ptmp.tile([64, 128], F32, tag="m4")
                    nc.vector.tensor_mul(m1, trp, tw[:, 0, :])
                    nc.vector.tensor_mul(m2, tip, tw[:, 1, :])
                    nc.vector.tensor_mul(m3, trp, tw[:, 1, :])
                    nc.vector.tensor_mul(m4, tip, tw[:, 0, :])
                    nc.vector.tensor_sub(tr_sb[:, b * 128:(b + 1) * 128], m1, m2)
                    nc.vector.tensor_add(ti_sb[:, b * 128:(b + 1) * 128], m3, m4)

                # stage 2: out[t, (b,q)] = c2^T Tr + (-s2)^T Ti
                ops = pif.tile([64, BPC * 128], F32, tag="ops")
                nc.tensor.matmul(ops, w2[:, 0, :], tr_sb, start=True, stop=False)
                nc.tensor.matmul(ops, w2[:, 1, :], ti_sb, start=False, stop=True)
                res = pifs.tile([64, BPC * 128], F32, tag="res")
                nc.scalar.copy(res, ops)
                for b in range(BPC):
                    nc.sync.dma_start(
                        out_d[b].rearrange("(t q) -> t q", q=128),
                        res[:, b * 128:(b + 1) * 128])

    nc.compile()
    return nc


def _host_consts(rand_s_1, rand_s_2, rand_h_1, rand_h_2):
    k = np.arange(KP)
    alpha = np.where((k == 0) | (k == D // 2), 1.0, 2.0) / D
    alpha = np.where(k > D // 2, 0.0, alpha)
    live = (k <= D // 2).astype(np.float64)
    s1 = rand_s_1.astype(np.float64)
    s2 = rand_s_2.astype(np.float64)
    th1 = 2.0 * np.pi * ((rand_h_1.astype(np.int64)[:, None] * k[None, :]) % D) / D
    th2 = 2.0 * np.pi * ((rand_h_2.astype(np.int64)[:, None] * k[None, :]) % D) / D
    A = np.empty((4, C, KP), np.float32)
    A[0] = s1[:, None] * np.cos(th1) * alpha
    A[1] = -s1[:, None] * np.sin(th1) * alpha
    A[2] = s2[:, None] * np.cos(th2) * live
    A[3] = -s2[:, None] * np.sin(th2) * live
    # amat layout [p, tensor, cc, kt, 128]
    amat = np.ascontiguousarray(
        A.reshape(4, CCN, 128, KT, 128).transpose(2, 0, 1, 3, 4))
    amat = amat.astype(ml_dtypes.bfloat16)

    # stage-1 IDFT weights, rows permuted to a' = p_hi*64 + kt <-> a = 2*kt + p_hi
    q = np.arange(128)[None, :]
    ap = np.arange(128)[:, None]          # a' index
    kt_of = ap % 64
    ph_of = ap // 64
    a_of = 2 * kt_of + ph_of
    valid = (kt_of < KT).astype(np.float64)
    c1 = np.cos(2 * np.pi * a_of * q / 128) * valid
    s1m = np.sin(2 * np.pi * a_of * q / 128) * valid
    w1 = np.stack([c1, s1m, -s1m], 1).astype(np.float32)  # [128, 3, 128]
    r_ = np.arange(64)[:, None]
    t_ = np.arange(64)[None, :]
    c2 = np.cos(2 * np.pi * t_ * r_ / 64)
    s2m = np.sin(2 * np.pi * t_ * r_ / 64)
    w2 = np.stack([c2, -s2m], 1).astype(np.float32)       # [64, 2, 64]
    ctw = np.cos(2 * np.pi * q * r_ / D)
    stw = np.sin(2 * np.pi * q * r_ / D)
    tw = np.stack([ctw, stw], 1).astype(np.float32)       # [64, 2, 128]
    ident = np.eye(128, dtype=np.float32)
    return amat, w1, w2, tw, ident


_NC_CACHE = None
LAST_RESULTS = None


def kernel(bottom1, bottom2, rand_s_1, rand_s_2, rand_h_1, rand_h_2):
    global _NC_CACHE
    if _NC_CACHE is None:
        _NC_CACHE = _build_nc()
    nc = _NC_CACHE

    amat, w1, w2, tw, ident = _host_consts(
        np.asarray(rand_s_1), np.asarray(rand_s_2),
        np.asarray(rand_h_1), np.asarray(rand_h_2))

    x1 = np.asarray(bottom1, np.float32).reshape(B, HW, C)
    x2 = np.asarray(bottom2, np.float32).reshape(B, HW, C)

    in_maps = []
    for core in range(NCORES):
        bs = slice(core * BPC, (core + 1) * BPC)
        xt = np.empty((2, C, ROWS), np.float32)
        xt[0] = x1[bs].reshape(ROWS, C).T
        xt[1] = x2[bs].reshape(ROWS, C).T
        xt = np.ascontiguousarray(
            xt.reshape(2, CCN, 128, ROWS).transpose(2, 0, 1, 3))
        xt = xt.astype(ml_dtypes.bfloat16)
        in_maps.append({
            "xt": xt, "amat": amat,
            "w1": w1, "w2": w2, "tw": tw, "ident": ident,
        })

    res = run_bass_kernel_spmd(nc, in_maps, core_ids=list(range(NCORES)))
    global LAST_RESULTS
    LAST_RESULTS = res
    out = np.concatenate([res.results[c]["out"] for c in range(NCORES)], 0)
    return out.astype(np.float32)


if __name__ == "__main__":
    rng = np.random.default_rng(0)
    b1 = rng.standard_normal((B, Hh, Ww, C)).astype(np.float32)
    b2 = rng.standard_normal((B, Hh, Ww, C)).astype(np.float32)
    s1 = (2.0 * rng.integers(0, 2, C) - 1.0).astype(np.float32)
    s2 = (2.0 * rng.integers(0, 2, C) - 1.0).astype(np.float32)
    h1 = rng.integers(0, D, C, dtype=np.int32)
    h2 = rng.integers(0, D, C, dtype=np.int32)
    out = kernel(bottom1=b1, bottom2=b2, rand_s_1=s1, rand_s_2=s2,
                 rand_h_1=h1, rand_h_2=h2)
    print(out.shape, out.dtype)


# revision 15
# speedup vs baseline: 1.0024x; 1.0024x over previous
"""Compact Bilinear Pooling (count-sketch + FFT circular correlation) as a
Trainium2 Bass kernel, data-parallel over batch across 8 NeuronCores.

Math: FFT(count_sketch(x; s, h))[k] = sum_c x[c] * s[c] * exp(-2pi i h[c] k / D)
    = x @ A, a dense complex matrix built on the host from (s, h). So the whole
layer is: Y1 = X1 @ A1, Y2 = X2 @ A2 (per-row half spectra), elementwise
complex product fused with the 14x14 sum-pool (DVE tensor_tensor_reduce),
then a real inverse FFT of the pooled [4, D] spectrum per core, done as a
two-stage Cooley-Tukey factorization (D = 64*128) of small matmuls.

The big spectrum matmuls run in bf16 (fast weight load + half the HBM
traffic); accumulation stays fp32 in PSUM. The pooled spectrum is reassembled
for the IFFT with PE transposes (no DRAM bounce).
"""
import numpy as np
import ml_dtypes

import concourse.bass as bass
import concourse.tile as tile
from concourse import bacc, mybir
from concourse.bass_utils import run_bass_kernel_spmd

B, Hh, Ww, C, D = 32, 14, 14, 512, 8192
NCORES = 8
BPC = B // NCORES        # 4 batches per core
HW = Hh * Ww             # 196
ROWS = BPC * HW          # 784 rows per core
KT = 33                  # frequency tiles of 128
KP = KT * 128            # 4224 >= D/2 + 1
CCN = 4                  # contraction chunks (C = 4*128)

F32 = mybir.dt.float32
BF16 = mybir.dt.bfloat16


def _build_nc():
    nc = bacc.Bacc("TRN2", target_bir_lowering=False)

    xt_d = nc.dram_tensor("xt", [128, 2, CCN, ROWS], BF16, kind="ExternalInput")
    amat_d = nc.dram_tensor("amat", [128, 4, CCN, KT, 128], BF16, kind="ExternalInput")
    w1_d = nc.dram_tensor("w1", [128, 3, 128], F32, kind="ExternalInput")
    w2_d = nc.dram_tensor("w2", [64, 2, 64], F32, kind="ExternalInput")
    tw_d = nc.dram_tensor("tw", [64, 2, 128], F32, kind="ExternalInput")
    id_d = nc.dram_tensor("ident", [128, 128], F32, kind="ExternalInput")
    out_d = nc.dram_tensor("out", [BPC, D], F32, kind="ExternalOutput")

    with tile.TileContext(nc) as tc:
        with tc.tile_pool(name="const", bufs=1) as pc, \
             tc.tile_pool(name="astream", bufs=3) as pa, \
             tc.tile_pool(name="ysbp", bufs=3) as pysb, \
             tc.tile_pool(name="scr", bufs=4) as pscr, \
             tc.tile_pool(name="qstage", bufs=1) as pqs:

            xt = pc.tile([128, 2, CCN, ROWS], BF16)
            for cc in range(CCN):
                for inp in range(2):
                    nc.sync.dma_start(xt[:, inp, cc], xt_d[:, inp, cc])
            qsb = pc.tile([128, KT, 2, 4, 2], F32)  # (kt, rc, term, seg)
            qsb_f = qsb  # noqa
            w1 = pc.tile([128, 3, 128], F32)
            nc.sync.dma_start(w1, w1_d[:, :, :])
            w2 = pc.tile([64, 2, 64], F32)
            nc.sync.dma_start(w2, w2_d[:, :, :])
            tw = pc.tile([64, 2, 128], F32)
            nc.sync.dma_start(tw, tw_d[:, :, :])
            ident = pc.tile([128, 128], F32)
            nc.sync.dma_start(ident, id_d[:, :])

            # ---------------- main loop ----------------
            with tc.tile_pool(name="py", bufs=2, space="PSUM") as py:
                for kt in range(KT):
                    at = pa.tile([128, 4, CCN, 128], BF16, tag="amat")
                    for t_ in range(4):
                        nc.sync.dma_start(at[:, t_], amat_d[:, t_, :, kt, :])
                    for rc in range(2):
                        # two 2-bank PSUM tiles: [re at 0:392 | im at 512:904]
                        yps = {}
                        for inp in range(2):
                            yps[inp] = py.tile([128, 1024], F32, tag=f"y{inp}",
                                               name=f"y{inp}")
                        for cc in range(CCN):
                            for inp in range(2):
                                for ri in range(2):
                                    nc.tensor.matmul(
                                        yps[inp][:, ri * 512:ri * 512 + 392],
                                        at[:, inp * 2 + ri, cc, :],
                                        xt[:, inp, cc, rc * 392:(rc + 1) * 392],
                                        start=(cc == 0),
                                        stop=(cc == CCN - 1),
                                    )
                        # one strided ACT copy per input: PSUM fp32 -> SBUF bf16
                        ysb = {}
                        for inp in range(2):
                            ysb[inp] = pysb.tile([128, 2, 392], BF16,
                                                 tag=f"ysb{inp}", name=f"ysb{inp}")
                            nc.scalar.copy(
                                ysb[inp],
                                yps[inp].rearrange("p (s x) -> p s x", s=2)[:, :, :392])
                        # products on DVE at 2x (bf16 SBUF x bf16 SBUF)
                        # (0: U=y1r*y2r, 1: V=y1i*y2i, 2: T1=y1r*y2i, 3: T2=y1i*y2r)
                        prods = pscr.tile([128, 4, 392], BF16, tag="prods",
                                          name="prods")
                        for term, (i0, i1) in enumerate([(0, 0), (1, 1), (0, 1), (1, 0)]):
                            nc.vector.tensor_mul(prods[:, term],
                                                 ysb[0][:, i0], ysb[1][:, i1])
                        # pooling: GPSIMD pre-folds 196 -> 98, then one
                        # multi-output free-axis reduce on DVE
                        pv = prods.rearrange("p t (s x) -> p t s x", s=2)
                        pref = pscr.tile([128, 4, 2, 98], F32, tag="pref",
                                         name="pref")
                        nc.gpsimd.tensor_tensor(
                            pref, pv[:, :, :, 0:98], pv[:, :, :, 98:196],
                            mybir.AluOpType.add)
                        nc.vector.tensor_reduce(
                            qsb[:, kt, rc].rearrange("p t s -> p (t s)"),
                            pref.rearrange("p t s x -> p (t s) x"),
                            axis=mybir.AxisListType.X, op=mybir.AluOpType.add)

            # combine terms: Qr = U - V, Qi = T1 + T2
            qstf = pqs.tile([128, KT, 2, 4], F32, tag="qstf")
            qstf_v = qstf.rearrange("p kt t (rc s) -> p kt t rc s", rc=2)
            nc.vector.tensor_sub(qstf_v[:, :, 0], qsb[:, :, :, 0, :], qsb[:, :, :, 1, :])
            nc.vector.tensor_add(qstf_v[:, :, 1], qsb[:, :, :, 2, :], qsb[:, :, :, 3, :])

            # ---------------- inverse FFT tail ----------------
            with tc.tile_pool(name="pif", bufs=1, space="PSUM") as pif, \
                 tc.tile_pool(name="ptr", bufs=2, space="PSUM") as ptr, \
                 tc.tile_pool(name="ptw", bufs=1, space="PSUM") as ptw, \
                 tc.tile_pool(name="pifs", bufs=1) as pifs, \
                 tc.tile_pool(name="ptmp", bufs=2) as ptmp:
                # reassemble Q for stage 1 with PE transposes (no DRAM bounce):
                # qar[a', b*64 + r] = Q[k] with k = kt*128 + p, p = p_hi*64 + r,
                # a' = p_hi*64 + kt (w1 rows are host-permuted to match; rows
                # with kt >= 33 are zero so garbage SBUF there is harmless).
                qar = pifs.tile([128, BPC * 64], F32, tag="qar")
                qai = pifs.tile([128, BPC * 64], F32, tag="qai")
                nc.vector.memset(qar, 0.0)
                nc.vector.memset(qai, 0.0)
                for t, dst in ((0, qar), (1, qai)):
                    for b in range(BPC):
                        tp = ptr.tile([33, 128], F32, tag="tp")
                        nc.tensor.transpose(tp, qstf[:, :, t, b], ident)
                        for ph in range(2):
                            nc.scalar.copy(
                                dst[ph * 64:ph * 64 + 33, b * 64:(b + 1) * 64],
                                tp[:, ph * 64:(ph + 1) * 64])

                # stage 1: V[q, (b,r)] ; contraction over a'
                vr_ps = pif.tile([128, BPC * 64], F32, tag="vr")
                vi_ps = pif.tile([128, BPC * 64], F32, tag="vi")
                nc.tensor.matmul(vr_ps, w1[:, 0, :], qar, start=True, stop=False)
                nc.tensor.matmul(vr_ps, w1[:, 2, :], qai, start=False, stop=True)
                nc.tensor.matmul(vi_ps, w1[:, 1, :], qar, start=True, stop=False)
                nc.tensor.matmul(vi_ps, w1[:, 0, :], qai, start=False, stop=True)
                vr_sb = pifs.tile([128, BPC * 64], F32, tag="vr_sb")
                vi_sb = pifs.tile([128, BPC * 64], F32, tag="vi_sb")
                nc.scalar.copy(vr_sb, vr_ps)
                nc.scalar.copy(vi_sb, vi_ps)

                # per-b transpose [128 q, 64 r] -> [64 r, 128 q], then twiddle
                tr_sb = pifs.tile([64, BPC * 128], F32, tag="tr_sb")
                ti_sb = pifs.tile([64, BPC * 128], F32, tag="ti_sb")
                for b in range(BPC):
                    trp = ptw.tile([64, 128], F32, tag="trp")
                    tip = ptw.tile([64, 128], F32, tag="tip")
                    nc.tensor.transpose(trp, vr_sb[:, b * 64:(b + 1) * 64], ident)
                    nc.tensor.transpose(tip, vi_sb[:, b * 64:(b + 1) * 64], ident)
                    m1 = ptmp.tile([64, 128], F32, tag="m1")
                    m2 = ptmp.tile([64, 128], F32, tag="m2")
                    m3 = ptmp.tile([64, 128], F32, tag="m3")
                    m4 = ptmp.tile([64, 128], F32, tag="m4")
                    nc.vector.tensor_mul(m1, trp, tw[:, 0, :])
                    nc.vector.tensor_mul(m2, tip, tw[:, 1, :])
                    nc.vector.tensor_mul(m3, trp, tw[:, 1, :])
                    nc.vector.tensor_mul(m4, tip, tw[:, 0, :])
                    nc.vector.tensor_sub(tr_sb[:, b * 128:(b + 1) * 128], m1, m2)
                    nc.vector.tensor_add(ti_sb[:, b * 128:(b + 1) * 128], m3, m4)

                # stage 2: out[t, (b,q)] = c2^T Tr + (-s2)^T Ti
                ops = pif.tile([64, BPC * 128], F32, tag="ops")
                nc.tensor.matmul(ops, w2[:, 0, :], tr_sb, start=True, stop=False)
                nc.tensor.matmul(ops, w2[:, 1, :], ti_sb, start=False, stop=True)
                res = pifs.tile([64, BPC * 128], F32, tag="res")
                nc.scalar.copy(res, ops)
                for b in range(BPC):
                    nc.sync.dma_start(
                        out_d[b].rearrange("(t q) -> t q", q=128),
                        res[:, b * 128:(b + 1) * 128])

    nc.compile()
    return nc


def _host_consts(rand_s_1, rand_s_2, rand_h_1, rand_h_2):
    k = np.arange(KP)
    alpha = np.where((k == 0) | (k == D // 2), 1.0, 2.0) / D
    alpha = np.where(k > D // 2, 0.0, alpha)
    live = (k <= D // 2).astype(np.float64)
    s1 = rand_s_1.astype(np.float64)
    s2 = rand_s_2.astype(np.float64)
    th1 = 2.0 * np.pi * ((rand_h_1.astype(np.int64)[:, None] * k[None, :]) % D) / D
    th2 = 2.0 * np.pi * ((rand_h_2.astype(np.int64)[:, None] * k[None, :]) % D) / D
    A = np.empty((4, C, KP), np.float32)
    A[0] = s1[:, None] * np.cos(th1) * alpha
    A[1] = -s1[:, None] * np.sin(th1) * alpha
    A[2] = s2[:, None] * np.cos(th2) * live
    A[3] = -s2[:, None] * np.sin(th2) * live
    # amat layout [p, tensor, cc, kt, 128]
    amat = np.ascontiguousarray(
        A.reshape(4, CCN, 128, KT, 128).transpose(2, 0, 1, 3, 4))
    amat = amat.astype(ml_dtypes.bfloat16)

    # stage-1 IDFT weights, rows permuted to a' = p_hi*64 + kt <-> a = 2*kt + p_hi
    q = np.arange(128)[None, :]
    ap = np.arange(128)[:, None]          # a' index
    kt_of = ap % 64
    ph_of = ap // 64
    a_of = 2 * kt_of + ph_of
    valid = (kt_of < KT).astype(np.float64)
    c1 = np.cos(2 * np.pi * a_of * q / 128) * valid
    s1m = np.sin(2 * np.pi * a_of * q / 128) * valid
    w1 = np.stack([c1, s1m, -s1m], 1).astype(np.float32)  # [128, 3, 128]
    r_ = np.arange(64)[:, None]
    t_ = np.arange(64)[None, :]
    c2 = np.cos(2 * np.pi * t_ * r_ / 64)
    s2m = np.sin(2 * np.pi * t_ * r_ / 64)
    w2 = np.stack([c2, -s2m], 1).astype(np.float32)       # [64, 2, 64]
    ctw = np.cos(2 * np.pi * q * r_ / D)
    stw = np.sin(2 * np.pi * q * r_ / D)
    tw = np.stack([ctw, stw], 1).astype(np.float32)       # [64, 2, 128]
    ident = np.eye(128, dtype=np.float32)
    return amat, w1, w2, tw, ident


_NC_CACHE = None
LAST_RESULTS = None


def kernel(bottom1, bottom2, rand_s_1, rand_s_2, rand_h_1, rand_h_2):
    global _NC_CACHE
    if _NC_CACHE is None:
        _NC_CACHE = _build_nc()
    nc = _NC_CACHE

    amat, w1, w2, tw, ident = _host_consts(
        np.asarray(rand_s_1), np.asarray(rand_s_2),
        np.asarray(rand_h_1), np.asarray(rand_h_2))

    x1 = np.asarray(bottom1, np.float32).reshape(B, HW, C)
    x2 = np.asarray(bottom2, np.float32).reshape(B, HW, C)

    in_maps = []
    for core in range(NCORES):
        bs = slice(core * BPC, (core + 1) * BPC)
        xt = np.empty((2, C, ROWS), np.float32)
        xt[0] = x1[bs].reshape(ROWS, C).T
        xt[1] = x2[bs].reshape(ROWS, C).T
        xt = np.ascontiguousarray(
            xt.reshape(2, CCN, 128, ROWS).transpose(2, 0, 1, 3))
        xt = xt.astype(ml_dtypes.bfloat16)
        in_maps.append({
            "xt": xt, "amat": amat,
            "w1": w1, "w2": w2, "tw": tw, "ident": ident,
        })

    res = run_bass_kernel_spmd(nc, in_maps, core_ids=list(range(NCORES)))
    global LAST_RESULTS
    LAST_RESULTS = res
    out = np.concatenate([res.results[c]["out"] for c in range(NCORES)], 0)
    return out.astype(np.float32)


if __name__ == "__main__":
    rng = np.random.default_rng(0)
    b1 = rng.standard_normal((B, Hh, Ww, C)).astype(np.float32)
    b2 = rng.standard_normal((B, Hh, Ww, C)).astype(np.float32)
    s1 = (2.0 * rng.integers(0, 2, C) - 1.0).astype(np.float32)
    s2 = (2.0 * rng.integers(0, 2, C) - 1.0).astype(np.float32)
    h1 = rng.integers(0, D, C, dtype=np.int32)
    h2 = rng.integers(0, D, C, dtype=np.int32)
    out = kernel(bottom1=b1, bottom2=b2, rand_s_1=s1, rand_s_2=s2,
                 rand_h_1=h1, rand_h_2=h2)
    print(out.shape, out.dtype)


# revision 16
# speedup vs baseline: 1.2020x; 1.1991x over previous
"""Compact Bilinear Pooling (count-sketch + FFT circular correlation) as a
Trainium2 Bass kernel, data-parallel over batch across 8 NeuronCores.

Math: FFT(count_sketch(x; s, h))[k] = sum_c x[c] * s[c] * exp(-2pi i h[c] k / D)
    = x @ A, a dense complex matrix built on the host from (s, h). So the whole
layer is: Y1 = X1 @ A1, Y2 = X2 @ A2 (per-row half spectra), elementwise
complex product fused with the 14x14 sum-pool (DVE tensor_tensor_reduce),
then a real inverse FFT of the pooled [4, D] spectrum per core, done as a
two-stage Cooley-Tukey factorization (D = 64*128) of small matmuls.

The big spectrum matmuls run in bf16 (fast weight load + half the HBM
traffic); accumulation stays fp32 in PSUM. The pooled spectrum is reassembled
for the IFFT with PE transposes (no DRAM bounce).
"""
import numpy as np
import ml_dtypes

import concourse.bass as bass
import concourse.tile as tile
from concourse import bacc, mybir
from concourse.bass_utils import run_bass_kernel_spmd

B, Hh, Ww, C, D = 32, 14, 14, 512, 8192
NCORES = 8
BPC = B // NCORES        # 4 batches per core
HW = Hh * Ww             # 196
ROWS = BPC * HW          # 784 rows per core
KT = 33                  # frequency tiles of 128
KP = KT * 128            # 4224 >= D/2 + 1
CCN = 4                  # contraction chunks (C = 4*128)

F32 = mybir.dt.float32
BF16 = mybir.dt.bfloat16


def _build_nc():
    nc = bacc.Bacc("TRN2", target_bir_lowering=False)

    xt_d = nc.dram_tensor("xt", [128, 2, CCN, ROWS], BF16, kind="ExternalInput")
    amat_d = nc.dram_tensor("amat", [128, 4, CCN, KT, 128], BF16, kind="ExternalInput")
    w1_d = nc.dram_tensor("w1", [128, 3, 128], F32, kind="ExternalInput")
    w2_d = nc.dram_tensor("w2", [64, 2, 64], F32, kind="ExternalInput")
    tw_d = nc.dram_tensor("tw", [64, 2, 128], F32, kind="ExternalInput")
    id_d = nc.dram_tensor("ident", [128, 128], F32, kind="ExternalInput")
    out_d = nc.dram_tensor("out", [BPC, D], F32, kind="ExternalOutput")

    with tile.TileContext(nc) as tc:
        with tc.tile_pool(name="const", bufs=1) as pc, \
             tc.tile_pool(name="astream", bufs=3) as pa, \
             tc.tile_pool(name="ysbp", bufs=3) as pysb, \
             tc.tile_pool(name="scr", bufs=4) as pscr, \
             tc.tile_pool(name="qstage", bufs=1) as pqs:

            xt = pc.tile([128, 2, CCN, ROWS], BF16)
            for cc in range(CCN):
                for inp in range(2):
                    nc.sync.dma_start(xt[:, inp, cc], xt_d[:, inp, cc])
            qsb = pc.tile([128, KT, 2, 4, 2], F32)  # (kt, rc, term, seg)
            qsb_f = qsb  # noqa
            w1 = pc.tile([128, 3, 128], F32)
            nc.sync.dma_start(w1, w1_d[:, :, :])
            w2 = pc.tile([64, 2, 64], F32)
            nc.sync.dma_start(w2, w2_d[:, :, :])
            tw = pc.tile([64, 2, 128], F32)
            nc.sync.dma_start(tw, tw_d[:, :, :])
            ident = pc.tile([128, 128], F32)
            nc.sync.dma_start(ident, id_d[:, :])

            # ---------------- main loop ----------------
            with tc.tile_pool(name="py", bufs=2, space="PSUM") as py:
                for kt in range(KT):
                    at = pa.tile([128, 4, CCN, 128], BF16, tag="amat")
                    for t_ in range(4):
                        nc.sync.dma_start(at[:, t_], amat_d[:, t_, :, kt, :])
                    for rc in range(2):
                        # two 2-bank PSUM tiles: [re at 0:392 | im at 512:904]
                        yps = {}
                        for inp in range(2):
                            yps[inp] = py.tile([128, 1024], F32, tag=f"y{inp}",
                                               name=f"y{inp}")
                        for cc in range(CCN):
                            for inp in range(2):
                                for ri in range(2):
                                    nc.tensor.matmul(
                                        yps[inp][:, ri * 512:ri * 512 + 392],
                                        at[:, inp * 2 + ri, cc, :],
                                        xt[:, inp, cc, rc * 392:(rc + 1) * 392],
                                        start=(cc == 0),
                                        stop=(cc == CCN - 1),
                                    )
                        # one strided ACT copy per input: PSUM fp32 -> SBUF bf16
                        ysb = {}
                        for inp in range(2):
                            ysb[inp] = pysb.tile([128, 2, 392], BF16,
                                                 tag=f"ysb{inp}", name=f"ysb{inp}")
                            nc.scalar.copy(
                                ysb[inp],
                                yps[inp].rearrange("p (s x) -> p s x", s=2)[:, :, :392])
                        # products on DVE at 2x (bf16 SBUF x bf16 SBUF)
                        # (0: U=y1r*y2r, 1: V=y1i*y2i, 2: T1=y1r*y2i, 3: T2=y1i*y2r)
                        prods = pscr.tile([128, 4, 392], BF16, tag="prods",
                                          name="prods")
                        for term, (i0, i1) in enumerate([(0, 0), (1, 1), (0, 1), (1, 0)]):
                            nc.vector.tensor_mul(prods[:, term],
                                                 ysb[0][:, i0], ysb[1][:, i1])
                        # pooling: one multi-output free-axis reduce (DVE)
                        nc.vector.tensor_reduce(
                            qsb[:, kt, rc].rearrange("p t s -> p (t s)"),
                            prods.rearrange("p t (s x) -> p (t s) x", s=2),
                            axis=mybir.AxisListType.X, op=mybir.AluOpType.add)

            # combine terms: Qr = U - V, Qi = T1 + T2
            qstf = pqs.tile([128, KT, 2, 4], F32, tag="qstf")
            qstf_v = qstf.rearrange("p kt t (rc s) -> p kt t rc s", rc=2)
            nc.vector.tensor_sub(qstf_v[:, :, 0], qsb[:, :, :, 0, :], qsb[:, :, :, 1, :])
            nc.vector.tensor_add(qstf_v[:, :, 1], qsb[:, :, :, 2, :], qsb[:, :, :, 3, :])

            # ---------------- inverse FFT tail ----------------
            with tc.tile_pool(name="pif", bufs=1, space="PSUM") as pif, \
                 tc.tile_pool(name="ptr", bufs=2, space="PSUM") as ptr, \
                 tc.tile_pool(name="ptw", bufs=1, space="PSUM") as ptw, \
                 tc.tile_pool(name="pifs", bufs=1) as pifs, \
                 tc.tile_pool(name="ptmp", bufs=2) as ptmp:
                # reassemble Q for stage 1 with PE transposes (no DRAM bounce):
                # qar[a', b*64 + r] = Q[k] with k = kt*128 + p, p = p_hi*64 + r,
                # a' = p_hi*64 + kt (w1 rows are host-permuted to match; rows
                # with kt >= 33 are zero so garbage SBUF there is harmless).
                qar = pifs.tile([128, BPC * 64], F32, tag="qar")
                qai = pifs.tile([128, BPC * 64], F32, tag="qai")
                nc.vector.memset(qar, 0.0)
                nc.vector.memset(qai, 0.0)
                for t, dst in ((0, qar), (1, qai)):
                    for b in range(BPC):
                        tp = ptr.tile([33, 128], F32, tag="tp")
                        nc.tensor.transpose(tp, qstf[:, :, t, b], ident)
                        for ph in range(2):
                            nc.scalar.copy(
                                dst[ph * 64:ph * 64 + 33, b * 64:(b + 1) * 64],
                                tp[:, ph * 64:(ph + 1) * 64])

                # stage 1: V[q, (b,r)] ; contraction over a'
                vr_ps = pif.tile([128, BPC * 64], F32, tag="vr")
                vi_ps = pif.tile([128, BPC * 64], F32, tag="vi")
                nc.tensor.matmul(vr_ps, w1[:, 0, :], qar, start=True, stop=False)
                nc.tensor.matmul(vr_ps, w1[:, 2, :], qai, start=False, stop=True)
                nc.tensor.matmul(vi_ps, w1[:, 1, :], qar, start=True, stop=False)
                nc.tensor.matmul(vi_ps, w1[:, 0, :], qai, start=False, stop=True)
                vr_sb = pifs.tile([128, BPC * 64], F32, tag="vr_sb")
                vi_sb = pifs.tile([128, BPC * 64], F32, tag="vi_sb")
                nc.scalar.copy(vr_sb, vr_ps)
                nc.scalar.copy(vi_sb, vi_ps)

                # per-b transpose [128 q, 64 r] -> [64 r, 128 q], then twiddle
                tr_sb = pifs.tile([64, BPC * 128], F32, tag="tr_sb")
                ti_sb = pifs.tile([64, BPC * 128], F32, tag="ti_sb")
                for b in range(BPC):
                    trp = ptw.tile([64, 128], F32, tag="trp")
                    tip = ptw.tile([64, 128], F32, tag="tip")
                    nc.tensor.transpose(trp, vr_sb[:, b * 64:(b + 1) * 64], ident)
                    nc.tensor.transpose(tip, vi_sb[:, b * 64:(b + 1) * 64], ident)
                    m1 = ptmp.tile([64, 128], F32, tag="m1")
                    m2 = ptmp.tile([64, 128], F32, tag="m2")
                    m3 = ptmp.tile([64, 128], F32, tag="m3")
                    m4 = ptmp.tile([64, 128], F32, tag="m4")
                    nc.vector.tensor_mul(m1, trp, tw[:, 0, :])
                    nc.vector.tensor_mul(m2, tip, tw[:, 1, :])
                    nc.vector.tensor_mul(m3, trp, tw[:, 1, :])
                    nc.vector.tensor_mul(m4, tip, tw[:, 0, :])
                    nc.vector.tensor_sub(tr_sb[:, b * 128:(b + 1) * 128], m1, m2)
                    nc.vector.tensor_add(ti_sb[:, b * 128:(b + 1) * 128], m3, m4)

                # stage 2: out[t, (b,q)] = c2^T Tr + (-s2)^T Ti
                ops = pif.tile([64, BPC * 128], F32, tag="ops")
                nc.tensor.matmul(ops, w2[:, 0, :], tr_sb, start=True, stop=False)
                nc.tensor.matmul(ops, w2[:, 1, :], ti_sb, start=False, stop=True)
                res = pifs.tile([64, BPC * 128], F32, tag="res")
                nc.scalar.copy(res, ops)
                for b in range(BPC):
                    nc.sync.dma_start(
                        out_d[b].rearrange("(t q) -> t q", q=128),
                        res[:, b * 128:(b + 1) * 128])

    nc.compile()
    return nc


def _host_consts(rand_s_1, rand_s_2, rand_h_1, rand_h_2):
    k = np.arange(KP)
    alpha = np.where((k == 0) | (k == D // 2), 1.0, 2.0) / D
    alpha = np.where(k > D // 2, 0.0, alpha)
    live = (k <= D // 2).astype(np.float64)
    s1 = rand_s_1.astype(np.float64)
    s2 = rand_s_2.astype(np.float64)
    th1 = 2.0 * np.pi * ((rand_h_1.astype(np.int64)[:, None] * k[None, :]) % D) / D
    th2 = 2.0 * np.pi * ((rand_h_2.astype(np.int64)[:, None] * k[None, :]) % D) / D
    A = np.empty((4, C, KP), np.float32)
    A[0] = s1[:, None] * np.cos(th1) * alpha
    A[1] = -s1[:, None] * np.sin(th1) * alpha
    A[2] = s2[:, None] * np.cos(th2) * live
    A[3] = -s2[:, None] * np.sin(th2) * live
    # amat layout [p, tensor, cc, kt, 128]
    amat = np.ascontiguousarray(
        A.reshape(4, CCN, 128, KT, 128).transpose(2, 0, 1, 3, 4))
    amat = amat.astype(ml_dtypes.bfloat16)

    # stage-1 IDFT weights, rows permuted to a' = p_hi*64 + kt <-> a = 2*kt + p_hi
    q = np.arange(128)[None, :]
    ap = np.arange(128)[:, None]          # a' index
    kt_of = ap % 64
    ph_of = ap // 64
    a_of = 2 * kt_of + ph_of
    valid = (kt_of < KT).astype(np.float64)
    c1 = np.cos(2 * np.pi * a_of * q / 128) * valid
    s1m = np.sin(2 * np.pi * a_of * q / 128) * valid
    w1 = np.stack([c1, s1m, -s1m], 1).astype(np.float32)  # [128, 3, 128]
    r_ = np.arange(64)[:, None]
    t_ = np.arange(64)[None, :]
    c2 = np.cos(2 * np.pi * t_ * r_ / 64)
    s2m = np.sin(2 * np.pi * t_ * r_ / 64)
    w2 = np.stack([c2, -s2m], 1).astype(np.float32)       # [64, 2, 64]
    ctw = np.cos(2 * np.pi * q * r_ / D)
    stw = np.sin(2 * np.pi * q * r_ / D)
    tw = np.stack([ctw, stw], 1).astype(np.float32)       # [64, 2, 128]
    ident = np.eye(128, dtype=np.float32)
    return amat, w1, w2, tw, ident


_NC_CACHE = None
LAST_RESULTS = None


def kernel(bottom1, bottom2, rand_s_1, rand_s_2, rand_h_1, rand_h_2):
    global _NC_CACHE
    if _NC_CACHE is None:
        _NC_CACHE = _build_nc()
    nc = _NC_CACHE

    amat, w1, w2, tw, ident = _host_consts(
        np.asarray(rand_s_1), np.asarray(rand_s_2),
        np.asarray(rand_h_1), np.asarray(rand_h_2))

    x1 = np.asarray(bottom1, np.float32).reshape(B, HW, C)
    x2 = np.asarray(bottom2, np.float32).reshape(B, HW, C)

    in_maps = []
    for core in range(NCORES):
        bs = slice(core * BPC, (core + 1) * BPC)
        xt = np.empty((2, C, ROWS), np.float32)
        xt[0] = x1[bs].reshape(ROWS, C).T
        xt[1] = x2[bs].reshape(ROWS, C).T
        xt = np.ascontiguousarray(
            xt.reshape(2, CCN, 128, ROWS).transpose(2, 0, 1, 3))
        xt = xt.astype(ml_dtypes.bfloat16)
        in_maps.append({
            "xt": xt, "amat": amat,
            "w1": w1, "w2": w2, "tw": tw, "ident": ident,
        })

    res = run_bass_kernel_spmd(nc, in_maps, core_ids=list(range(NCORES)))
    global LAST_RESULTS
    LAST_RESULTS = res
    out = np.concatenate([res.results[c]["out"] for c in range(NCORES)], 0)
    return out.astype(np.float32)


if __name__ == "__main__":
    rng = np.random.default_rng(0)
    b1 = rng.standard_normal((B, Hh, Ww, C)).astype(np.float32)
    b2 = rng.standard_normal((B, Hh, Ww, C)).astype(np.float32)
    s1 = (2.0 * rng.integers(0, 2, C) - 1.0).astype(np.float32)
    s2 = (2.0 * rng.integers(0, 2, C) - 1.0).astype(np.float32)
    h1 = rng.integers(0, D, C, dtype=np.int32)
    h2 = rng.integers(0, D, C, dtype=np.int32)
    out = kernel(bottom1=b1, bottom2=b2, rand_s_1=s1, rand_s_2=s2,
                 rand_h_1=h1, rand_h_2=h2)
    print(out.shape, out.dtype)


# revision 19
# speedup vs baseline: 1.2470x; 1.0375x over previous
"""Compact Bilinear Pooling (count-sketch + FFT circular correlation) as a
Trainium2 Bass kernel, data-parallel over batch across 8 NeuronCores.

Math: FFT(count_sketch(x; s, h))[k] = sum_c x[c] * s[c] * exp(-2pi i h[c] k / D)
    = x @ A, a dense complex matrix built on the host from (s, h). So the whole
layer is: Y1 = X1 @ A1, Y2 = X2 @ A2 (per-row half spectra), elementwise
complex product fused with the 14x14 sum-pool (DVE tensor_tensor_reduce),
then a real inverse FFT of the pooled [4, D] spectrum per core, done as a
two-stage Cooley-Tukey factorization (D = 64*128) of small matmuls.

The big spectrum matmuls run in bf16 (fast weight load + half the HBM
traffic); accumulation stays fp32 in PSUM. The pooled spectrum is reassembled
for the IFFT with PE transposes (no DRAM bounce).
"""
import numpy as np
import ml_dtypes

import concourse.bass as bass
import concourse.tile as tile
from concourse import bacc, mybir
from concourse.bass_utils import run_bass_kernel_spmd

B, Hh, Ww, C, D = 32, 14, 14, 512, 8192
NCORES = 8
BPC = B // NCORES        # 4 batches per core
HW = Hh * Ww             # 196
ROWS = BPC * HW          # 784 rows per core
KT = 33                  # frequency tiles of 128
KP = KT * 128            # 4224 >= D/2 + 1
CCN = 4                  # contraction chunks (C = 4*128)

F32 = mybir.dt.float32
BF16 = mybir.dt.bfloat16


def _build_nc():
    nc = bacc.Bacc("TRN2", target_bir_lowering=False)

    xt_d = nc.dram_tensor("xt", [128, 2, CCN, ROWS], BF16, kind="ExternalInput")
    amat_d = nc.dram_tensor("amat", [128, 4, CCN, KT, 128], BF16, kind="ExternalInput")
    w1_d = nc.dram_tensor("w1", [128, 3, 128], F32, kind="ExternalInput")
    w2_d = nc.dram_tensor("w2", [64, 2, 64], F32, kind="ExternalInput")
    tw_d = nc.dram_tensor("tw", [64, 2, 128], F32, kind="ExternalInput")
    id_d = nc.dram_tensor("ident", [128, 128], F32, kind="ExternalInput")
    out_d = nc.dram_tensor("out", [BPC, D], F32, kind="ExternalOutput")

    with tile.TileContext(nc) as tc:
        with tc.tile_pool(name="const", bufs=1) as pc, \
             tc.tile_pool(name="astream", bufs=3) as pa, \
             tc.tile_pool(name="ysbp", bufs=3) as pysb, \
             tc.tile_pool(name="scr", bufs=4) as pscr, \
             tc.tile_pool(name="qstage", bufs=1) as pqs:

            xt = pc.tile([128, 2, CCN, ROWS], BF16)
            for cc in range(CCN):
                for inp in range(2):
                    nc.sync.dma_start(xt[:, inp, cc], xt_d[:, inp, cc])
            qsb = pc.tile([128, KT, 2, 4, 2], F32)  # (kt, rc, term, seg)
            qsb_f = qsb  # noqa
            w1 = pc.tile([128, 3, 128], F32)
            nc.sync.dma_start(w1, w1_d[:, :, :])
            w2 = pc.tile([64, 2, 64], F32)
            nc.sync.dma_start(w2, w2_d[:, :, :])
            tw = pc.tile([64, 2, 128], F32)
            nc.sync.dma_start(tw, tw_d[:, :, :])
            ident = pc.tile([128, 128], F32)
            nc.sync.dma_start(ident, id_d[:, :])

            # ---------------- main loop ----------------
            with tc.tile_pool(name="py", bufs=2, space="PSUM") as py:
                for kt in range(KT):
                    at = pa.tile([128, 4, CCN, 128], BF16, tag="amat")
                    for t_ in range(4):
                        nc.sync.dma_start(at[:, t_], amat_d[:, t_, :, kt, :])
                    for rc in range(2):
                        # two 2-bank PSUM tiles: [re at 0:392 | im at 512:904]
                        yps = {}
                        for inp in range(2):
                            yps[inp] = py.tile([128, 1024], F32, tag=f"y{inp}",
                                               name=f"y{inp}")
                        for cc in range(CCN):
                            for inp in range(2):
                                for ri in range(2):
                                    nc.tensor.matmul(
                                        yps[inp][:, ri * 512:ri * 512 + 392],
                                        at[:, inp * 2 + ri, cc, :],
                                        xt[:, inp, cc, rc * 392:(rc + 1) * 392],
                                        start=(cc == 0),
                                        stop=(cc == CCN - 1),
                                    )
                        # one strided ACT copy per input: PSUM fp32 -> SBUF bf16
                        ysb = {}
                        for inp in range(2):
                            ysb[inp] = pysb.tile([128, 2, 392], BF16,
                                                 tag=f"ysb{inp}", name=f"ysb{inp}")
                            nc.scalar.copy(
                                ysb[inp],
                                yps[inp].rearrange("p (s x) -> p s x", s=2)[:, :, :392])
                        # all 4 products in ONE DVE op via broadcast APs,
                        # term order (0:U=y1r*y2r, 1:T1=y1r*y2i, 2:T2=y1i*y2r,
                        # 3:V=y1i*y2i): y1 pattern (r,r,i,i), y2 (r,i,r,i)
                        prods = pscr.tile([128, 4, 392], BF16, tag="prods",
                                          name="prods")
                        in0 = ysb[0][:, :, :].unsqueeze(2) \
                            .broadcast_to([128, 2, 2, 392])
                        in1 = ysb[1][:, :, :].unsqueeze(1) \
                            .broadcast_to([128, 2, 2, 392])
                        nc.vector.tensor_mul(
                            prods.rearrange("p (i j) x -> p i j x", i=2),
                            in0, in1)
                        # pooling: 2x-mode pairwise fold 196 -> 98, then one
                        # multi-output free-axis reduce
                        pv = prods.rearrange("p t (s x) -> p t s x", s=2)
                        pref = pscr.tile([128, 4, 2, 98], BF16, tag="pref",
                                         name="pref")
                        nc.vector.tensor_add(pref, pv[:, :, :, 0:98],
                                             pv[:, :, :, 98:196])
                        nc.vector.tensor_reduce(
                            qsb[:, kt, rc].rearrange("p t s -> p (t s)"),
                            pref.rearrange("p t s x -> p (t s) x"),
                            axis=mybir.AxisListType.X, op=mybir.AluOpType.add)

            # combine terms: Qr = U - V, Qi = T1 + T2
            qstf = pqs.tile([128, KT, 2, 4], F32, tag="qstf")
            qstf_v = qstf.rearrange("p kt t (rc s) -> p kt t rc s", rc=2)
            nc.vector.tensor_sub(qstf_v[:, :, 0], qsb[:, :, :, 0, :], qsb[:, :, :, 3, :])
            nc.vector.tensor_add(qstf_v[:, :, 1], qsb[:, :, :, 1, :], qsb[:, :, :, 2, :])

            # ---------------- inverse FFT tail ----------------
            with tc.tile_pool(name="pif", bufs=1, space="PSUM") as pif, \
                 tc.tile_pool(name="ptr", bufs=2, space="PSUM") as ptr, \
                 tc.tile_pool(name="ptw", bufs=1, space="PSUM") as ptw, \
                 tc.tile_pool(name="pifs", bufs=1) as pifs, \
                 tc.tile_pool(name="ptmp", bufs=2) as ptmp:
                # reassemble Q for stage 1 with PE transposes (no DRAM bounce):
                # qar[a', b*64 + r] = Q[k] with k = kt*128 + p, p = p_hi*64 + r,
                # a' = p_hi*64 + kt (w1 rows are host-permuted to match; rows
                # with kt >= 33 are zero so garbage SBUF there is harmless).
                qar = pifs.tile([128, BPC * 64], F32, tag="qar")
                qai = pifs.tile([128, BPC * 64], F32, tag="qai")
                nc.vector.memset(qar, 0.0)
                nc.vector.memset(qai, 0.0)
                for t, dst in ((0, qar), (1, qai)):
                    for b in range(BPC):
                        tp = ptr.tile([33, 128], F32, tag="tp")
                        nc.tensor.transpose(tp, qstf[:, :, t, b], ident)
                        for ph in range(2):
                            nc.scalar.copy(
                                dst[ph * 64:ph * 64 + 33, b * 64:(b + 1) * 64],
                                tp[:, ph * 64:(ph + 1) * 64])

                # stage 1: V[q, (b,r)] ; contraction over a'
                vr_ps = pif.tile([128, BPC * 64], F32, tag="vr")
                vi_ps = pif.tile([128, BPC * 64], F32, tag="vi")
                nc.tensor.matmul(vr_ps, w1[:, 0, :], qar, start=True, stop=False)
                nc.tensor.matmul(vr_ps, w1[:, 2, :], qai, start=False, stop=True)
                nc.tensor.matmul(vi_ps, w1[:, 1, :], qar, start=True, stop=False)
                nc.tensor.matmul(vi_ps, w1[:, 0, :], qai, start=False, stop=True)
                vr_sb = pifs.tile([128, BPC * 64], F32, tag="vr_sb")
                vi_sb = pifs.tile([128, BPC * 64], F32, tag="vi_sb")
                nc.scalar.copy(vr_sb, vr_ps)
                nc.scalar.copy(vi_sb, vi_ps)

                # per-b transpose [128 q, 64 r] -> [64 r, 128 q], then twiddle
                tr_sb = pifs.tile([64, BPC * 128], F32, tag="tr_sb")
                ti_sb = pifs.tile([64, BPC * 128], F32, tag="ti_sb")
                for b in range(BPC):
                    trp = ptw.tile([64, 128], F32, tag="trp")
                    tip = ptw.tile([64, 128], F32, tag="tip")
                    nc.tensor.transpose(trp, vr_sb[:, b * 64:(b + 1) * 64], ident)
                    nc.tensor.transpose(tip, vi_sb[:, b * 64:(b + 1) * 64], ident)
                    m1 = ptmp.tile([64, 128], F32, tag="m1")
                    m2 = ptmp.tile([64, 128], F32, tag="m2")
                    m3 = ptmp.tile([64, 128], F32, tag="m3")
                    m4 = ptmp.tile([64, 128], F32, tag="m4")
                    nc.vector.tensor_mul(m1, trp, tw[:, 0, :])
                    nc.vector.tensor_mul(m2, tip, tw[:, 1, :])
                    nc.vector.tensor_mul(m3, trp, tw[:, 1, :])
                    nc.vector.tensor_mul(m4, tip, tw[:, 0, :])
                    nc.vector.tensor_sub(tr_sb[:, b * 128:(b + 1) * 128], m1, m2)
                    nc.vector.tensor_add(ti_sb[:, b * 128:(b + 1) * 128], m3, m4)

                # stage 2: out[t, (b,q)] = c2^T Tr + (-s2)^T Ti
                ops = pif.tile([64, BPC * 128], F32, tag="ops")
                nc.tensor.matmul(ops, w2[:, 0, :], tr_sb, start=True, stop=False)
                nc.tensor.matmul(ops, w2[:, 1, :], ti_sb, start=False, stop=True)
                res = pifs.tile([64, BPC * 128], F32, tag="res")
                nc.scalar.copy(res, ops)
                for b in range(BPC):
                    nc.sync.dma_start(
                        out_d[b].rearrange("(t q) -> t q", q=128),
                        res[:, b * 128:(b + 1) * 128])

    nc.compile()
    return nc


def _host_consts(rand_s_1, rand_s_2, rand_h_1, rand_h_2):
    k = np.arange(KP)
    alpha = np.where((k == 0) | (k == D // 2), 1.0, 2.0) / D
    alpha = np.where(k > D // 2, 0.0, alpha)
    live = (k <= D // 2).astype(np.float64)
    s1 = rand_s_1.astype(np.float64)
    s2 = rand_s_2.astype(np.float64)
    th1 = 2.0 * np.pi * ((rand_h_1.astype(np.int64)[:, None] * k[None, :]) % D) / D
    th2 = 2.0 * np.pi * ((rand_h_2.astype(np.int64)[:, None] * k[None, :]) % D) / D
    A = np.empty((4, C, KP), np.float32)
    A[0] = s1[:, None] * np.cos(th1) * alpha
    A[1] = -s1[:, None] * np.sin(th1) * alpha
    A[2] = s2[:, None] * np.cos(th2) * live
    A[3] = -s2[:, None] * np.sin(th2) * live
    # amat layout [p, tensor, cc, kt, 128]
    amat = np.ascontiguousarray(
        A.reshape(4, CCN, 128, KT, 128).transpose(2, 0, 1, 3, 4))
    amat = amat.astype(ml_dtypes.bfloat16)

    # stage-1 IDFT weights, rows permuted to a' = p_hi*64 + kt <-> a = 2*kt + p_hi
    q = np.arange(128)[None, :]
    ap = np.arange(128)[:, None]          # a' index
    kt_of = ap % 64
    ph_of = ap // 64
    a_of = 2 * kt_of + ph_of
    valid = (kt_of < KT).astype(np.float64)
    c1 = np.cos(2 * np.pi * a_of * q / 128) * valid
    s1m = np.sin(2 * np.pi * a_of * q / 128) * valid
    w1 = np.stack([c1, s1m, -s1m], 1).astype(np.float32)  # [128, 3, 128]
    r_ = np.arange(64)[:, None]
    t_ = np.arange(64)[None, :]
    c2 = np.cos(2 * np.pi * t_ * r_ / 64)
    s2m = np.sin(2 * np.pi * t_ * r_ / 64)
    w2 = np.stack([c2, -s2m], 1).astype(np.float32)       # [64, 2, 64]
    ctw = np.cos(2 * np.pi * q * r_ / D)
    stw = np.sin(2 * np.pi * q * r_ / D)
    tw = np.stack([ctw, stw], 1).astype(np.float32)       # [64, 2, 128]
    ident = np.eye(128, dtype=np.float32)
    return amat, w1, w2, tw, ident


_NC_CACHE = None
LAST_RESULTS = None


def kernel(bottom1, bottom2, rand_s_1, rand_s_2, rand_h_1, rand_h_2):
    global _NC_CACHE
    if _NC_CACHE is None:
        _NC_CACHE = _build_nc()
    nc = _NC_CACHE

    amat, w1, w2, tw, ident = _host_consts(
        np.asarray(rand_s_1), np.asarray(rand_s_2),
        np.asarray(rand_h_1), np.asarray(rand_h_2))

    x1 = np.asarray(bottom1, np.float32).reshape(B, HW, C)
    x2 = np.asarray(bottom2, np.float32).reshape(B, HW, C)

    in_maps = []
    for core in range(NCORES):
        bs = slice(core * BPC, (core + 1) * BPC)
        xt = np.empty((2, C, ROWS), np.float32)
        xt[0] = x1[bs].reshape(ROWS, C).T
        xt[1] = x2[bs].reshape(ROWS, C).T
        xt = np.ascontiguousarray(
            xt.reshape(2, CCN, 128, ROWS).transpose(2, 0, 1, 3))
        xt = xt.astype(ml_dtypes.bfloat16)
        in_maps.append({
            "xt": xt, "amat": amat,
            "w1": w1, "w2": w2, "tw": tw, "ident": ident,
        })

    res = run_bass_kernel_spmd(nc, in_maps, core_ids=list(range(NCORES)))
    global LAST_RESULTS
    LAST_RESULTS = res
    out = np.concatenate([res.results[c]["out"] for c in range(NCORES)], 0)
    return out.astype(np.float32)


if __name__ == "__main__":
    rng = np.random.default_rng(0)
    b1 = rng.standard_normal((B, Hh, Ww, C)).astype(np.float32)
    b2 = rng.standard_normal((B, Hh, Ww, C)).astype(np.float32)
    s1 = (2.0 * rng.integers(0, 2, C) - 1.0).astype(np.float32)
    s2 = (2.0 * rng.integers(0, 2, C) - 1.0).astype(np.float32)
    h1 = rng.integers(0, D, C, dtype=np.int32)
    h2 = rng.integers(0, D, C, dtype=np.int32)
    out = kernel(bottom1=b1, bottom2=b2, rand_s_1=s1, rand_s_2=s2,
                 rand_h_1=h1, rand_h_2=h2)
    print(out.shape, out.dtype)


# revision 20
# speedup vs baseline: 1.2927x; 1.0366x over previous
"""Compact Bilinear Pooling (count-sketch + FFT circular correlation) as a
Trainium2 Bass kernel, data-parallel over batch across 8 NeuronCores.

Math: FFT(count_sketch(x; s, h))[k] = sum_c x[c] * s[c] * exp(-2pi i h[c] k / D)
    = x @ A, a dense complex matrix built on the host from (s, h). So the whole
layer is: Y1 = X1 @ A1, Y2 = X2 @ A2 (per-row half spectra), elementwise
complex product fused with the 14x14 sum-pool (DVE tensor_tensor_reduce),
then a real inverse FFT of the pooled [4, D] spectrum per core, done as a
two-stage Cooley-Tukey factorization (D = 64*128) of small matmuls.

The big spectrum matmuls run in bf16 (fast weight load + half the HBM
traffic); accumulation stays fp32 in PSUM. The pooled spectrum is reassembled
for the IFFT with PE transposes (no DRAM bounce).
"""
import numpy as np
import ml_dtypes

import concourse.bass as bass
import concourse.tile as tile
from concourse import bacc, mybir
from concourse.bass_utils import run_bass_kernel_spmd

B, Hh, Ww, C, D = 32, 14, 14, 512, 8192
NCORES = 8
BPC = B // NCORES        # 4 batches per core
HW = Hh * Ww             # 196
ROWS = BPC * HW          # 784 rows per core
KT = 33                  # frequency tiles of 128
KP = KT * 128            # 4224 >= D/2 + 1
CCN = 4                  # contraction chunks (C = 4*128)

F32 = mybir.dt.float32
BF16 = mybir.dt.bfloat16


def _build_nc():
    nc = bacc.Bacc("TRN2", target_bir_lowering=False)

    xt_d = nc.dram_tensor("xt", [128, 2, CCN, ROWS], BF16, kind="ExternalInput")
    amat_d = nc.dram_tensor("amat", [128, 4, CCN, KT, 128], BF16, kind="ExternalInput")
    w1_d = nc.dram_tensor("w1", [128, 3, 128], BF16, kind="ExternalInput")
    w2_d = nc.dram_tensor("w2", [64, 2, 64], BF16, kind="ExternalInput")
    tw_d = nc.dram_tensor("tw", [64, 2, 128], F32, kind="ExternalInput")
    id_d = nc.dram_tensor("ident", [128, 128], F32, kind="ExternalInput")
    out_d = nc.dram_tensor("out", [BPC, D], F32, kind="ExternalOutput")

    with tile.TileContext(nc) as tc:
        with tc.tile_pool(name="const", bufs=1) as pc, \
             tc.tile_pool(name="astream", bufs=3) as pa, \
             tc.tile_pool(name="ysbp", bufs=3) as pysb, \
             tc.tile_pool(name="scr", bufs=4) as pscr, \
             tc.tile_pool(name="qstage", bufs=1) as pqs:

            xt = pc.tile([128, 2, CCN, ROWS], BF16)
            for cc in range(CCN):
                for inp in range(2):
                    nc.scalar.dma_start(xt[:, inp, cc], xt_d[:, inp, cc])
            qsb = pc.tile([128, KT, 2, 4, 2], F32)  # (kt, rc, term, seg)
            qsb_f = qsb  # noqa
            w1 = pc.tile([128, 3, 128], BF16)
            nc.scalar.dma_start(w1, w1_d[:, :, :])
            w2 = pc.tile([64, 2, 64], BF16)
            nc.scalar.dma_start(w2, w2_d[:, :, :])
            tw = pc.tile([64, 2, 128], F32)
            nc.scalar.dma_start(tw, tw_d[:, :, :])
            ident = pc.tile([128, 128], F32)
            nc.scalar.dma_start(ident, id_d[:, :])

            # ---------------- main loop ----------------
            with tc.tile_pool(name="py", bufs=2, space="PSUM") as py:
                for kt in range(KT):
                    at = pa.tile([128, 4, CCN, 128], BF16, tag="amat")
                    for t_ in range(4):
                        nc.sync.dma_start(at[:, t_], amat_d[:, t_, :, kt, :])
                    for rc in range(2):
                        # two 2-bank PSUM tiles: [re at 0:392 | im at 512:904]
                        yps = {}
                        for inp in range(2):
                            yps[inp] = py.tile([128, 1024], F32, tag=f"y{inp}",
                                               name=f"y{inp}")
                        for cc in range(CCN):
                            for inp in range(2):
                                for ri in range(2):
                                    nc.tensor.matmul(
                                        yps[inp][:, ri * 512:ri * 512 + 392],
                                        at[:, inp * 2 + ri, cc, :],
                                        xt[:, inp, cc, rc * 392:(rc + 1) * 392],
                                        start=(cc == 0),
                                        stop=(cc == CCN - 1),
                                    )
                        # one strided ACT copy per input: PSUM fp32 -> SBUF bf16
                        ysb = {}
                        for inp in range(2):
                            ysb[inp] = pysb.tile([128, 2, 392], BF16,
                                                 tag=f"ysb{inp}", name=f"ysb{inp}")
                            nc.scalar.copy(
                                ysb[inp],
                                yps[inp].rearrange("p (s x) -> p s x", s=2)[:, :, :392])
                        # all 4 products in ONE DVE op via broadcast APs,
                        # term order (0:U=y1r*y2r, 1:T1=y1r*y2i, 2:T2=y1i*y2r,
                        # 3:V=y1i*y2i): y1 pattern (r,r,i,i), y2 (r,i,r,i)
                        prods = pscr.tile([128, 4, 392], BF16, tag="prods",
                                          name="prods")
                        in0 = ysb[0][:, :, :].unsqueeze(2) \
                            .broadcast_to([128, 2, 2, 392])
                        in1 = ysb[1][:, :, :].unsqueeze(1) \
                            .broadcast_to([128, 2, 2, 392])
                        nc.vector.tensor_mul(
                            prods.rearrange("p (i j) x -> p i j x", i=2),
                            in0, in1)
                        # pooling: 2x-mode pairwise fold 196 -> 98, then one
                        # multi-output free-axis reduce
                        pv = prods.rearrange("p t (s x) -> p t s x", s=2)
                        pref = pscr.tile([128, 4, 2, 98], BF16, tag="pref",
                                         name="pref")
                        nc.vector.tensor_add(pref, pv[:, :, :, 0:98],
                                             pv[:, :, :, 98:196])
                        pref2 = pscr.tile([128, 4, 2, 49], BF16, tag="pref2",
                                          name="pref2")
                        nc.vector.tensor_add(pref2, pref[:, :, :, 0:49],
                                             pref[:, :, :, 49:98])
                        nc.vector.tensor_reduce(
                            qsb[:, kt, rc].rearrange("p t s -> p (t s)"),
                            pref2.rearrange("p t s x -> p (t s) x"),
                            axis=mybir.AxisListType.X, op=mybir.AluOpType.add)

            # combine terms: Qr = U - V, Qi = T1 + T2
            qstf = pqs.tile([128, KT, 2, 4], F32, tag="qstf")
            qstf_v = qstf.rearrange("p kt t (rc s) -> p kt t rc s", rc=2)
            nc.vector.tensor_sub(qstf_v[:, :, 0], qsb[:, :, :, 0, :], qsb[:, :, :, 3, :])
            nc.vector.tensor_add(qstf_v[:, :, 1], qsb[:, :, :, 1, :], qsb[:, :, :, 2, :])

            # ---------------- inverse FFT tail ----------------
            with tc.tile_pool(name="pif", bufs=1, space="PSUM") as pif, \
                 tc.tile_pool(name="ptr", bufs=2, space="PSUM") as ptr, \
                 tc.tile_pool(name="ptw", bufs=1, space="PSUM") as ptw, \
                 tc.tile_pool(name="pifs", bufs=1) as pifs, \
                 tc.tile_pool(name="ptmp", bufs=2) as ptmp:
                # reassemble Q for stage 1 with PE transposes (no DRAM bounce):
                # qar[a', b*64 + r] = Q[k] with k = kt*128 + p, p = p_hi*64 + r,
                # a' = p_hi*64 + kt (w1 rows are host-permuted to match; rows
                # with kt >= 33 are zero so garbage SBUF there is harmless).
                qar = pifs.tile([128, BPC * 64], BF16, tag="qar")
                qai = pifs.tile([128, BPC * 64], BF16, tag="qai")
                nc.vector.memset(qar, 0.0)
                nc.vector.memset(qai, 0.0)
                for t, dst in ((0, qar), (1, qai)):
                    for b in range(BPC):
                        tp = ptr.tile([33, 128], F32, tag="tp")
                        nc.tensor.transpose(tp, qstf[:, :, t, b], ident)
                        for ph in range(2):
                            nc.scalar.copy(
                                dst[ph * 64:ph * 64 + 33, b * 64:(b + 1) * 64],
                                tp[:, ph * 64:(ph + 1) * 64])

                # stage 1: V[q, (b,r)] ; contraction over a'
                vr_ps = pif.tile([128, BPC * 64], F32, tag="vr")
                vi_ps = pif.tile([128, BPC * 64], F32, tag="vi")
                nc.tensor.matmul(vr_ps, w1[:, 0, :], qar, start=True, stop=False)
                nc.tensor.matmul(vr_ps, w1[:, 2, :], qai, start=False, stop=True)
                nc.tensor.matmul(vi_ps, w1[:, 1, :], qar, start=True, stop=False)
                nc.tensor.matmul(vi_ps, w1[:, 0, :], qai, start=False, stop=True)
                vr_sb = pifs.tile([128, BPC * 64], F32, tag="vr_sb")
                vi_sb = pifs.tile([128, BPC * 64], F32, tag="vi_sb")
                nc.scalar.copy(vr_sb, vr_ps)
                nc.scalar.copy(vi_sb, vi_ps)

                # per-b transpose [128 q, 64 r] -> [64 r, 128 q], then twiddle
                tr_sb = pifs.tile([64, BPC * 128], BF16, tag="tr_sb")
                ti_sb = pifs.tile([64, BPC * 128], BF16, tag="ti_sb")
                for b in range(BPC):
                    trp = ptw.tile([64, 128], F32, tag="trp")
                    tip = ptw.tile([64, 128], F32, tag="tip")
                    nc.tensor.transpose(trp, vr_sb[:, b * 64:(b + 1) * 64], ident)
                    nc.tensor.transpose(tip, vi_sb[:, b * 64:(b + 1) * 64], ident)
                    m1 = ptmp.tile([64, 128], F32, tag="m1")
                    m2 = ptmp.tile([64, 128], F32, tag="m2")
                    m3 = ptmp.tile([64, 128], F32, tag="m3")
                    m4 = ptmp.tile([64, 128], F32, tag="m4")
                    nc.vector.tensor_mul(m1, trp, tw[:, 0, :])
                    nc.vector.tensor_mul(m2, tip, tw[:, 1, :])
                    nc.vector.tensor_mul(m3, trp, tw[:, 1, :])
                    nc.vector.tensor_mul(m4, tip, tw[:, 0, :])
                    nc.vector.tensor_sub(tr_sb[:, b * 128:(b + 1) * 128], m1, m2)
                    nc.vector.tensor_add(ti_sb[:, b * 128:(b + 1) * 128], m3, m4)

                # stage 2: out[t, (b,q)] = c2^T Tr + (-s2)^T Ti
                ops = pif.tile([64, BPC * 128], F32, tag="ops")
                nc.tensor.matmul(ops, w2[:, 0, :], tr_sb, start=True, stop=False)
                nc.tensor.matmul(ops, w2[:, 1, :], ti_sb, start=False, stop=True)
                res = pifs.tile([64, BPC * 128], F32, tag="res")
                nc.scalar.copy(res, ops)
                for b in range(BPC):
                    nc.sync.dma_start(
                        out_d[b].rearrange("(t q) -> t q", q=128),
                        res[:, b * 128:(b + 1) * 128])

    nc.compile()
    return nc


def _host_consts(rand_s_1, rand_s_2, rand_h_1, rand_h_2):
    k = np.arange(KP)
    alpha = np.where((k == 0) | (k == D // 2), 1.0, 2.0) / D
    alpha = np.where(k > D // 2, 0.0, alpha)
    live = (k <= D // 2).astype(np.float64)
    s1 = rand_s_1.astype(np.float64)
    s2 = rand_s_2.astype(np.float64)
    th1 = 2.0 * np.pi * ((rand_h_1.astype(np.int64)[:, None] * k[None, :]) % D) / D
    th2 = 2.0 * np.pi * ((rand_h_2.astype(np.int64)[:, None] * k[None, :]) % D) / D
    A = np.empty((4, C, KP), np.float32)
    A[0] = s1[:, None] * np.cos(th1) * alpha
    A[1] = -s1[:, None] * np.sin(th1) * alpha
    A[2] = s2[:, None] * np.cos(th2) * live
    A[3] = -s2[:, None] * np.sin(th2) * live
    # amat layout [p, tensor, cc, kt, 128]
    amat = np.ascontiguousarray(
        A.reshape(4, CCN, 128, KT, 128).transpose(2, 0, 1, 3, 4))
    amat = amat.astype(ml_dtypes.bfloat16)

    # stage-1 IDFT weights, rows permuted to a' = p_hi*64 + kt <-> a = 2*kt + p_hi
    q = np.arange(128)[None, :]
    ap = np.arange(128)[:, None]          # a' index
    kt_of = ap % 64
    ph_of = ap // 64
    a_of = 2 * kt_of + ph_of
    valid = (kt_of < KT).astype(np.float64)
    c1 = np.cos(2 * np.pi * a_of * q / 128) * valid
    s1m = np.sin(2 * np.pi * a_of * q / 128) * valid
    w1 = np.stack([c1, s1m, -s1m], 1).astype(ml_dtypes.bfloat16)  # [128, 3, 128]
    r_ = np.arange(64)[:, None]
    t_ = np.arange(64)[None, :]
    c2 = np.cos(2 * np.pi * t_ * r_ / 64)
    s2m = np.sin(2 * np.pi * t_ * r_ / 64)
    w2 = np.stack([c2, -s2m], 1).astype(ml_dtypes.bfloat16)       # [64, 2, 64]
    ctw = np.cos(2 * np.pi * q * r_ / D)
    stw = np.sin(2 * np.pi * q * r_ / D)
    tw = np.stack([ctw, stw], 1).astype(np.float32)       # [64, 2, 128]
    ident = np.eye(128, dtype=np.float32)
    return amat, w1, w2, tw, ident


_NC_CACHE = None
LAST_RESULTS = None


def kernel(bottom1, bottom2, rand_s_1, rand_s_2, rand_h_1, rand_h_2):
    global _NC_CACHE
    if _NC_CACHE is None:
        _NC_CACHE = _build_nc()
    nc = _NC_CACHE

    amat, w1, w2, tw, ident = _host_consts(
        np.asarray(rand_s_1), np.asarray(rand_s_2),
        np.asarray(rand_h_1), np.asarray(rand_h_2))

    x1 = np.asarray(bottom1, np.float32).reshape(B, HW, C)
    x2 = np.asarray(bottom2, np.float32).reshape(B, HW, C)

    in_maps = []
    for core in range(NCORES):
        bs = slice(core * BPC, (core + 1) * BPC)
        xt = np.empty((2, C, ROWS), np.float32)
        xt[0] = x1[bs].reshape(ROWS, C).T
        xt[1] = x2[bs].reshape(ROWS, C).T
        xt = np.ascontiguousarray(
            xt.reshape(2, CCN, 128, ROWS).transpose(2, 0, 1, 3))
        xt = xt.astype(ml_dtypes.bfloat16)
        in_maps.append({
            "xt": xt, "amat": amat,
            "w1": w1, "w2": w2, "tw": tw, "ident": ident,
        })

    res = run_bass_kernel_spmd(nc, in_maps, core_ids=list(range(NCORES)))
    global LAST_RESULTS
    LAST_RESULTS = res
    out = np.concatenate([res.results[c]["out"] for c in range(NCORES)], 0)
    return out.astype(np.float32)


if __name__ == "__main__":
    rng = np.random.default_rng(0)
    b1 = rng.standard_normal((B, Hh, Ww, C)).astype(np.float32)
    b2 = rng.standard_normal((B, Hh, Ww, C)).astype(np.float32)
    s1 = (2.0 * rng.integers(0, 2, C) - 1.0).astype(np.float32)
    s2 = (2.0 * rng.integers(0, 2, C) - 1.0).astype(np.float32)
    h1 = rng.integers(0, D, C, dtype=np.int32)
    h2 = rng.integers(0, D, C, dtype=np.int32)
    out = kernel(bottom1=b1, bottom2=b2, rand_s_1=s1, rand_s_2=s2,
                 rand_h_1=h1, rand_h_2=h2)
    print(out.shape, out.dtype)
